# revision 19
# baseline (speedup 1.0000x reference)
"""Bass/Trainium2 kernel for nn_ABAgInteractionLayer (cross-attention + residual).

Sharding: data-parallel over batch B=8 -> one batch element per NeuronCore.
No collectives; each core computes its full batch slice.

Algebraic refactoring (host-side, weight-only constant folding):
  scores = (Xb Wq + bq)(Xg Wk + bk)^T / 16
         = Xb M Xg^T + (r 1^T) Xg^T + const_per_q      M = Wq Wk^T/16,
                                                       r = bq^T Wk^T/16
  (the const-per-q term from bk cancels in softmax)
  inter  = (P (Xg Wv + bv)) Wo + bo = P Xg N + (bv Wo + bo)   N = Wv Wo
  (P rows sum to 1, so bv contributes a constant vector folded into res)
So the device kernel never computes K or V projections: attention runs
directly against raw Xg, and only two small projections remain, both as
single fp8-DoubleRow matmuls:
  q8 = e4m3(0.25 * (Xb8 @ m8) + 16 r)       m8 = e4m3(64 M),  [f-major]
  sT[k,q] = k8 . q8                          e4m3 DoubleRow, [k,q] layout
  eT = exp(sT/16) as fp8e5m2, split across two engines per a 24/16
       pattern on SINGLE k-tiles ([128,512] PSUM bank granularity):
       ACT: native Exp activation (scale=1/16)
       DVE: one-pass Schraudolph: e5m2 bits are the top 8 bits of fp16,
            so int8(x*(4/ln2)/16 + 60) bitcast to e5m2 IS exp(x/16).
            The constant multiplicative bias cancels in the softmax ratio.
  AV[q,:] = sum_k eT[k,q] * vab[k,:]   (e5m2 DoubleRow; vab = [Xg | 1/16 |
                                        pad], col 256 accumulates Z/16)
  avn8 = e4m3(16 AV / Z)  -> PE transpose -> avnT8 [d,q] e4m3
  op   = avnT8 @ n8  (one fp8-DR matmul; n8 = e4m3(16 N))
  out  = op/256 + res    res = f16(Xb + bv Wo + bo), streamed per 128-row j

PSUM layout (8 banks, fully packed):
  - 6-bank "production ring" of [128,512] f32 slots shared, in PE program
    order, by scores singles / M-proj tiles / out-proj outputs.  Slot =
    counter % 6; all consumers (exp, convert, residual-add) drain within
    ~2 productions, so 6 slots hide worst-case exp queueing latency.
  - 2 AV banks (chain parity j%2): av+Z in [0:272], epilogue transpose
    scratch in [400:464].

Scheduling: flat global-step event list; scores+exp for q-block n+1
interleave with the AV chains + epilogue of q-block n.  Startup DMAs are
issued from five different engines (descriptor generation is ~650ns per
DMA and serializes per-engine) in need-order, so the first M-proj runs
~4us after the preamble barrier instead of ~13us.
"""

import sys

if "/opt/trn_rl_repo" not in sys.path:
    sys.path.insert(0, "/opt/trn_rl_repo")

import ml_dtypes
import numpy as np

import concourse.bacc as bacc
import concourse.bass as bass
import concourse.mybir as mybir
import concourse.tile as tile
from concourse import masks
from concourse.bass_utils import run_bass_kernel_spmd

B, L, A, F = 8, 512, 5, 256
H = 256
LQ = L * A          # 2560 query tokens
LK = 1024 * 5       # 5120 key tokens
NCORES = 8
QBLK = 512
NQB = LQ // QBLK    # 5
KT = 128
NKT = LK // KT      # 40 k-tiles per q-block
NKP = NKT // 2      # 20 k-tile pairs (AV chain contraction steps)
VW = 272            # Xg | 1/16 | pad (fp8 DoubleRow pair step must be %16)
RING = 6            # psum production-ring slots ([128,512] f32 = 1 bank)
DT = mybir.dt.float32
F16 = mybir.dt.float16
F8E4 = mybir.dt.float8e4
F8E5 = mybir.dt.float8e5
BF = mybir.dt.bfloat16
I8 = mybir.dt.int8
NP_F16 = np.float16
NP_E4 = ml_dtypes.float8_e4m3
NP_E5 = ml_dtypes.float8_e5m2

QSCALE = 16.0       # q8 = e4m3(16*q''); exp compensates with scale=1/16
MPS = 0.25          # mproj epilogue scale: psum = Xb @ (64 M) -> *16/64
# outproj scale chain: avn=bf16(av/256), n8=e4m3(16N) ->
# op = av.N/16; inter = av.N/Z = op*(16/Z) = op*reciprocal(z)
A_SC = (4.0 / np.log(2.0)) / QSCALE
B_SC = 60.0         # e5m2 exponent bias (15*4); DVE converts round-to-nearest

DR = mybir.MatmulPerfMode.DoubleRow


def _mk_pattern(n, na):
    # na 'A's among n singles, spread evenly (Bresenham)
    out, acc = [], 0
    for _ in range(n):
        acc += na
        if acc >= n:
            acc -= n
            out.append("A")
        else:
            out.append("D")
    return "".join(out)


# engine patterns are per k-tile PAIR: both singles of a pair write one
# exb tile, and same-engine writes stay program-ordered (a cross-engine
# write-write pair on one tile costs a semaphore hop).
PH1 = _mk_pattern(NKP, 11)  # phase-1: no epilogue -> 22A/18D singles
STD = _mk_pattern(NKP, 12)  # steady: 24A/16D leaves DVE epilogue slack


def build():
    nc = bacc.Bacc("TRN2", target_bir_lowering=False, debug=False,
                   num_devices=NCORES)
    xb8d = nc.dram_tensor("xb8d", [128, 2, LQ], F8E4, kind="ExternalInput")
    k8d = nc.dram_tensor("k8d", [128, 2, LK], F8E4, kind="ExternalInput")
    vabd = nc.dram_tensor("vabd", [128, NKP, 2, VW], F8E5,
                          kind="ExternalInput")
    resd = nc.dram_tensor("resd", [LQ, H], F16, kind="ExternalInput")
    m8d = nc.dram_tensor("m8d", [128, 2, 2, 128], F8E4, kind="ExternalInput")
    n8d = nc.dram_tensor("n8d", [128, 2, H], F8E4, kind="ExternalInput")
    rqd = nc.dram_tensor("rqd", [128, 2], DT, kind="ExternalInput")
    out = nc.dram_tensor("out", [LQ, H], F16, kind="ExternalOutput")

    ActF = mybir.ActivationFunctionType
    Alu = mybir.AluOpType

    with tile.TileContext(nc) as tc:
        with (
            tc.tile_pool(name="const", bufs=1) as cp,
            tc.tile_pool(name="persist", bufs=1) as pp,
            tc.tile_pool(name="ring", bufs=RING,
                         space=bass.MemorySpace.PSUM) as rgp,
            tc.tile_pool(name="avps", bufs=2,
                         space=bass.MemorySpace.PSUM) as avp,
            tc.tile_pool(name="exbufs", bufs=2 * NKP) as exp_pool,
            tc.tile_pool(name="epil", bufs=2) as elp,
        ):
            m8_s = cp.tile([128, 2, 2, 128], F8E4, tag="m8")
            n8_s = cp.tile([128, 2, H], F8E4, tag="n8")
            rq_s = cp.tile([128, 2], DT, tag="rq")
            zb = cp.tile([128, 1], DT, tag="zb")
            identb = cp.tile([128, 128], BF, tag="ident")
            k8 = pp.tile([128, 2, LK], F8E4, tag="k8")
            vab = pp.tile([128, NKP, 2, VW], F8E5, tag="vab")
            xb8 = pp.tile([128, 2, LQ], F8E4, tag="xb8")
            q8_b = [pp.tile([128, 2, QBLK], F8E4, tag=f"q8{t}", name=f"q8{t}")
                    for t in range(NQB)]

            # startup DMAs: descriptor-gen (~650ns each) serializes
            # per-engine, and only sync/scalar/gpsimd can initiate DMAs —
            # fan the critical ones out across all three in need-order:
            # mproj wants m8+xb8[0], scores want k8 (quartered so scores
            # tile t only waits on its quarter via subtile deps).
            nc.scalar.dma_start(m8_s[:], m8d[:])
            nc.scalar.dma_start(xb8[:, :, 0:QBLK], xb8d[:, :, 0:QBLK])
            nc.scalar.dma_start(rq_s[:], rqd[:])
            KQ = LK // 4
            for q in range(2):
                nc.gpsimd.dma_start(k8[:, :, q * KQ:(q + 1) * KQ],
                                    k8d[:, :, q * KQ:(q + 1) * KQ])
            nc.sync.dma_start(k8[:, :, 2 * KQ:3 * KQ],
                              k8d[:, :, 2 * KQ:3 * KQ])
            nc.sync.dma_start(k8[:, :, 3 * KQ:4 * KQ],
                              k8d[:, :, 3 * KQ:4 * KQ])
            vc = (0, 7, 14, NKP)
            for h in range(3):
                nc.sync.dma_start(vab[:, vc[h]:vc[h + 1], :, :],
                                  vabd[:, vc[h]:vc[h + 1], :, :])
            for t in range(1, NQB):
                nc.sync.dma_start(xb8[:, :, t * QBLK:(t + 1) * QBLK],
                                  xb8d[:, :, t * QBLK:(t + 1) * QBLK])
            nc.sync.dma_start(n8_s[:], n8d[:])
            nc.vector.memset(zb[:], 0.0)
            masks.make_identity(nc, identb[:])

            def next_slot():
                return rgp.tile([128, 512], DT, tag="ring", name="ring")[:]

            def mproj(t, engines):
                # q8[t][:, co, :] = e4m3(0.25*(m8[:,:,co].T @ xb8_t) + 16r)
                for co in range(2):
                    ps = next_slot()
                    nc.tensor.matmul(
                        ps, m8_s[:, :, co, :],
                        xb8[:, :, t * QBLK:(t + 1) * QBLK],
                        perf_mode=DR, start=True, stop=True)
                    dst = q8_b[t][:, co, :]
                    if engines[co] == "A":
                        nc.scalar.activation(dst, ps, ActF.Identity,
                                             bias=rq_s[:, co:co + 1],
                                             scale=MPS)
                    else:
                        nc.vector.tensor_scalar(dst, ps, MPS,
                                                rq_s[:, co:co + 1],
                                                Alu.mult, Alu.add)

            def scores_single(qb, t, exb_list, eng):
                ps = next_slot()
                nc.tensor.matmul(
                    ps, k8[:, :, t * KT:(t + 1) * KT], q8_b[qb][:, :, :],
                    perf_mode=DR, start=True, stop=True)
                if t % 2 == 0:
                    exb_list[t // 2] = exp_pool.tile([128, 2, QBLK], I8,
                                                     tag="exb", name="exb")
                dst = exb_list[t // 2][:, t % 2, :]
                if eng == "A":
                    nc.scalar.activation(dst.bitcast(F8E5), ps, ActF.Exp,
                                         bias=zb[:], scale=1.0 / QSCALE)
                else:
                    nc.vector.tensor_scalar(dst, ps, A_SC, B_SC,
                                            Alu.mult, Alu.add)

            def av_mms(j, kps, exb_list, avst):
                if 0 in kps:
                    avst["av"] = avp.tile([128, 512], DT, tag="av",
                                          name="av")
                av = avst["av"]
                for kp in kps:
                    nc.tensor.matmul(
                        av[:, 0:VW],
                        exb_list[kp][:, :, j * 128:(j + 1) * 128]
                        .bitcast(F8E5),
                        vab[:, kp, :, :],
                        perf_mode=DR,
                        start=(kp == 0), stop=(kp == NKP - 1))

            # Epilogue stages are scheduled 2 steps apart so each stage's
            # cross-engine inputs are produced >1us earlier: neither PE nor
            # DVE blocks at its queue head on a fresh dependency.
            def epi_s0(j, qb, st):
                # Z is ~constant across q (rel std ~2.4%), so convert av
                # with a FIXED 2^-8 scale; the exact per-row 1/Z lands in
                # the final residual-add as a per-partition scalar.  This
                # keeps reciprocal off the PE transpose's critical path.
                rows = qb * QBLK + j * 128
                av = st["avst"]["av"]
                rec = elp.tile([128, 1], DT, tag="rec")
                nc.vector.reciprocal(rec[:], av[:, 256:257])
                avn = elp.tile([128, H], BF, tag="avn")
                nc.vector.tensor_scalar_mul(avn[:], av[:, 0:H], 1.0 / 256.0)
                res_t = elp.tile([128, H], F16, tag="res")
                nc.sync.dma_start(res_t[:], resd[rows:rows + 128, :])
                st["avn"], st["res_t"], st["rec"] = avn, res_t, rec

            def epi_s1(j, qb, st):
                avn = st["avn"]
                avnT = elp.tile([128, 2, 128], F8E4, tag="avnT")
                tps = next_slot()
                for c in range(2):
                    tp = tps[:, 64 * c:64 * (c + 1)].bitcast(BF)
                    nc.tensor.transpose(tp, avn[:, c * 128:(c + 1) * 128],
                                        identb[:])
                for c in range(2):
                    tp = tps[:, 64 * c:64 * (c + 1)].bitcast(BF)
                    nc.scalar.copy(avnT[:, c, :], tp)
                st["avnT"] = avnT

            def epi_s2(j, qb, st):
                rows = qb * QBLK + j * 128
                op = next_slot()[:, 0:H]
                nc.tensor.matmul(op, st["avnT"][:, :, :], n8_s[:, :, :],
                                 perf_mode=DR, start=True, stop=True)
                out_t = elp.tile([128, H], F16, tag="out")
                nc.vector.scalar_tensor_tensor(out_t[:], op, st["rec"][:],
                                               st["res_t"], Alu.mult, Alu.add)
                nc.sync.dma_start(out[rows:rows + 128, :], out_t[:])

            # ---- phase 1: M-proj + scores/exp for q-block 0 ----
            exb_all = [[None] * NKP for _ in range(NQB)]
            mproj(0, "AD")
            mproj(1, "DA")
            for t in range(NKT):
                scores_single(0, t, exb_all[0], PH1[t // 2])

            # ---- main schedule: flat global-step event list ----
            events = []  # (g, prio, fn)
            for qb in range(NQB):
                g0 = qb * NKP
                if qb + 1 < NQB:
                    for s in range(NKP):
                        events.append((g0 + s, 0,
                                       lambda s=s, qb=qb:
                                       [scores_single(qb + 1, 2 * s + h,
                                                      exb_all[qb + 1],
                                                      STD[s])
                                        for h in range(2)]))
                if qb + 2 < NQB:
                    events.append((g0 + 1, 1,
                                   lambda qb=qb: mproj(qb + 2, "AD")))
                for j in range(4):
                    avst = {}
                    for s in range(5):
                        events.append((g0 + 5 * j + 2 + s, 2,
                                       lambda j=j, s=s, qb=qb, avst=avst:
                                       av_mms(j, range(s * 4, s * 4 + 4),
                                              exb_all[qb], avst)))
                    st = {"avst": avst}
                    # last window: exp engines are idle, so the epilogue's
                    # cross-engine stages need no queue-hiding stagger
                    d1, d2 = (1, 2) if qb == NQB - 1 else (2, 4)
                    events.append((g0 + 5 * j + 6, 3,
                                   lambda j=j, qb=qb, st=st:
                                   epi_s0(j, qb, st)))
                    events.append((g0 + 5 * j + 6 + d1, 3,
                                   lambda j=j, qb=qb, st=st:
                                   epi_s1(j, qb, st)))
                    events.append((g0 + 5 * j + 6 + d2, 3,
                                   lambda j=j, qb=qb, st=st:
                                   epi_s2(j, qb, st)))
            events.sort(key=lambda e: (e[0], e[1]))
            for _, _, fn in events:
                fn()

    nc.compile()
    return nc


_nc_cache = None
last_results = None


def _get_nc():
    global _nc_cache
    if _nc_cache is None:
        _nc_cache = build()
    return _nc_cache


def _fmajor(w, np_dt):
    # [256, X] -> [128, 2, X] with row c*128+p -> [p, c]
    return np.ascontiguousarray(
        w.reshape(2, 128, -1).transpose(1, 0, 2).astype(np_dt))


def _vab_layout(xg):
    # [5120, 256] -> [128, NKP, 2, VW] e5m2: token t = (kp*2+c)*128+p,
    # cols 0:256 = xg row, col 256 = 1/16 (so Z accumulates as Z/16).
    v = np.zeros((NKP, 2, 128, VW), dtype=NP_E5)
    v[:, :, :, 0:F] = xg.reshape(NKP, 2, 128, F).astype(NP_E5)
    v[:, :, :, F] = np.float32(1.0 / 16.0)
    return np.ascontiguousarray(v.transpose(2, 0, 1, 3))


def kernel(**inputs):
    global last_results
    ab = np.ascontiguousarray(inputs["ab"], dtype=np.float32)
    ag = np.ascontiguousarray(inputs["ag"], dtype=np.float32)
    Wq = np.asarray(inputs["Wq"], dtype=np.float32)
    Wk = np.asarray(inputs["Wk"], dtype=np.float32)
    Wv = np.asarray(inputs["Wv"], dtype=np.float32)
    Wo = np.asarray(inputs["Wo"], dtype=np.float32)
    bq = np.asarray(inputs["bq"], dtype=np.float32)
    bv = np.asarray(inputs["bv"], dtype=np.float32)
    bo = np.asarray(inputs["bo"], dtype=np.float32)

    s = np.float32(1.0 / np.sqrt(np.float32(H)))
    M = s * (Wq @ Wk.T)                          # [256 f, 256 d]
    N = Wv @ Wo                                  # [256 d, 256 f]
    r = s * (bq @ Wk.T)                          # [256 d]; bk cancels
    cvec = bv @ Wo + bo                          # [256 f]

    m8 = np.ascontiguousarray(                   # [p, ci, co, m]
        (64.0 * M).reshape(2, 128, 2, 128).transpose(1, 0, 2, 3)
        .astype(NP_E4))
    n8 = _fmajor(16.0 * N, NP_E4)                # [p, c, f]
    rqd = np.ascontiguousarray((QSCALE * r).reshape(2, 128).T,
                               dtype=np.float32)

    in_maps = []
    for b in range(B):
        xb = ab[b].reshape(LQ, F)
        xg = ag[b].reshape(LK, F)
        in_maps.append({
            "xb8d": _fmajor(xb.T, NP_E4),
            "k8d": _fmajor(xg.T, NP_E4),
            "vabd": _vab_layout(xg),
            "resd": np.ascontiguousarray((xb + cvec[None, :])
                                         .astype(NP_F16)),
            "m8d": m8, "n8d": n8, "rqd": rqd,
        })

    nc = _get_nc()
    last_exc = None
    for _attempt in range(3):
        try:
            last_results = run_bass_kernel_spmd(
                nc, in_maps, core_ids=list(range(NCORES)))
            break
        except Exception as e:  # transient device flakes
            last_exc = e
    else:
        raise last_exc
    return np.stack([last_results.results[b]["out"].reshape(L, A, F)
                     for b in range(B)]).astype(np.float32)


# revision 20
# speedup vs baseline: 1.0115x; 1.0115x over previous
"""Bass/Trainium2 kernel for nn_ABAgInteractionLayer (cross-attention + residual).

Sharding: data-parallel over batch B=8 -> one batch element per NeuronCore.
No collectives; each core computes its full batch slice.

Algebraic refactoring (host-side, weight-only constant folding):
  scores = (Xb Wq + bq)(Xg Wk + bk)^T / 16
         = Xb M Xg^T + (r 1^T) Xg^T + const_per_q      M = Wq Wk^T/16,
                                                       r = bq^T Wk^T/16
  (the const-per-q term from bk cancels in softmax)
  inter  = (P (Xg Wv + bv)) Wo + bo = P Xg N + (bv Wo + bo)   N = Wv Wo
  (P rows sum to 1, so bv contributes a constant vector folded into res)
So the device kernel never computes K or V projections: attention runs
directly against raw Xg, and only two small projections remain, both as
single fp8-DoubleRow matmuls:
  q8 = e4m3(0.25 * (Xb8 @ m8) + 16 r)       m8 = e4m3(64 M),  [f-major]
  sT[k,q] = k8 . q8                          e4m3 DoubleRow, [k,q] layout
  eT = exp(sT/16) as fp8e5m2, split across two engines per a 24/16
       pattern on SINGLE k-tiles ([128,512] PSUM bank granularity):
       ACT: native Exp activation (scale=1/16)
       DVE: one-pass Schraudolph: e5m2 bits are the top 8 bits of fp16,
            so int8(x*(4/ln2)/16 + 60) bitcast to e5m2 IS exp(x/16).
            The constant multiplicative bias cancels in the softmax ratio.
  AV[q,:] = sum_k eT[k,q] * vab[k,:]   (e5m2 DoubleRow; vab = [Xg | 1/16 |
                                        pad], col 256 accumulates Z/16)
  avn8 = e4m3(16 AV / Z)  -> PE transpose -> avnT8 [d,q] e4m3
  op   = avnT8 @ n8  (one fp8-DR matmul; n8 = e4m3(16 N))
  out  = op/256 + res    res = f16(Xb + bv Wo + bo), streamed per 128-row j

PSUM layout (8 banks, fully packed):
  - 6-bank "production ring" of [128,512] f32 slots shared, in PE program
    order, by scores singles / M-proj tiles / out-proj outputs.  Slot =
    counter % 6; all consumers (exp, convert, residual-add) drain within
    ~2 productions, so 6 slots hide worst-case exp queueing latency.
  - 2 AV banks (chain parity j%2): av+Z in [0:272], epilogue transpose
    scratch in [400:464].

Scheduling: flat global-step event list; scores+exp for q-block n+1
interleave with the AV chains + epilogue of q-block n.  Startup DMAs are
issued from five different engines (descriptor generation is ~650ns per
DMA and serializes per-engine) in need-order, so the first M-proj runs
~4us after the preamble barrier instead of ~13us.
"""

import sys

if "/opt/trn_rl_repo" not in sys.path:
    sys.path.insert(0, "/opt/trn_rl_repo")

import ml_dtypes
import numpy as np

import concourse.bacc as bacc
import concourse.bass as bass
import concourse.mybir as mybir
import concourse.tile as tile
from concourse import masks
from concourse.bass_utils import run_bass_kernel_spmd

B, L, A, F = 8, 512, 5, 256
H = 256
LQ = L * A          # 2560 query tokens
LK = 1024 * 5       # 5120 key tokens
NCORES = 8
QBLK = 512
NQB = LQ // QBLK    # 5
KT = 128
NKT = LK // KT      # 40 k-tiles per q-block
NKP = NKT // 2      # 20 k-tile pairs (AV chain contraction steps)
VW = 272            # Xg | 1/16 | pad (fp8 DoubleRow pair step must be %16)
RING = 6            # psum production-ring slots ([128,512] f32 = 1 bank)
DT = mybir.dt.float32
F16 = mybir.dt.float16
F8E4 = mybir.dt.float8e4
F8E5 = mybir.dt.float8e5
BF = mybir.dt.bfloat16
I8 = mybir.dt.int8
NP_F16 = np.float16
NP_E4 = ml_dtypes.float8_e4m3
NP_E5 = ml_dtypes.float8_e5m2

QSCALE = 16.0       # q8 = e4m3(16*q''); exp compensates with scale=1/16
MPS = 0.25          # mproj epilogue scale: psum = Xb @ (64 M) -> *16/64
# outproj scale chain: avn=bf16(av/256), n8=e4m3(16N) ->
# op = av.N/16; inter = av.N/Z = op*(16/Z) = op*reciprocal(z)
A_SC = (4.0 / np.log(2.0)) / QSCALE
B_SC = 60.0         # e5m2 exponent bias (15*4); DVE converts round-to-nearest

DR = mybir.MatmulPerfMode.DoubleRow


def _mk_pattern(n, na):
    # na 'A's among n singles, spread evenly (Bresenham)
    out, acc = [], 0
    for _ in range(n):
        acc += na
        if acc >= n:
            acc -= n
            out.append("A")
        else:
            out.append("D")
    return "".join(out)


# engine patterns are per k-tile PAIR: both singles of a pair write one
# exb tile, and same-engine writes stay program-ordered (a cross-engine
# write-write pair on one tile costs a semaphore hop).
PH1 = _mk_pattern(NKP, 11)  # phase-1: no epilogue -> 22A/18D singles
STD = _mk_pattern(NKP, 12)  # steady: 24A/16D leaves DVE epilogue slack


def build():
    nc = bacc.Bacc("TRN2", target_bir_lowering=False, debug=False,
                   num_devices=NCORES)
    xb8d = nc.dram_tensor("xb8d", [128, 2, LQ], F8E4, kind="ExternalInput")
    k8d = nc.dram_tensor("k8d", [128, 2, LK], F8E4, kind="ExternalInput")
    vabd = nc.dram_tensor("vabd", [128, NKP, 2, VW], F8E5,
                          kind="ExternalInput")
    resd = nc.dram_tensor("resd", [LQ, H], F16, kind="ExternalInput")
    m8d = nc.dram_tensor("m8d", [128, 2, 2, 128], F8E4, kind="ExternalInput")
    n8d = nc.dram_tensor("n8d", [128, 2, H], F8E4, kind="ExternalInput")
    rqd = nc.dram_tensor("rqd", [128, 2], DT, kind="ExternalInput")
    out = nc.dram_tensor("out", [LQ, H], F16, kind="ExternalOutput")

    ActF = mybir.ActivationFunctionType
    Alu = mybir.AluOpType

    with tile.TileContext(nc) as tc:
        with (
            tc.tile_pool(name="const", bufs=1) as cp,
            tc.tile_pool(name="persist", bufs=1) as pp,
            tc.tile_pool(name="ring", bufs=RING,
                         space=bass.MemorySpace.PSUM) as rgp,
            tc.tile_pool(name="avps", bufs=2,
                         space=bass.MemorySpace.PSUM) as avp,
            tc.tile_pool(name="exbufs", bufs=2 * NKP) as exp_pool,
            tc.tile_pool(name="epil", bufs=2) as elp,
        ):
            m8_s = cp.tile([128, 2, 2, 128], F8E4, tag="m8")
            n8_s = cp.tile([128, 2, H], F8E4, tag="n8")
            rq_s = cp.tile([128, 2], DT, tag="rq")
            zb = cp.tile([128, 1], DT, tag="zb")
            identb = cp.tile([128, 128], BF, tag="ident")
            k8 = pp.tile([128, 2, LK], F8E4, tag="k8")
            vab = pp.tile([128, NKP, 2, VW], F8E5, tag="vab")
            xb8 = pp.tile([128, 2, LQ], F8E4, tag="xb8")
            q8_b = [pp.tile([128, 2, QBLK], F8E4, tag=f"q8{t}", name=f"q8{t}")
                    for t in range(NQB)]

            # startup DMAs: descriptor-gen (~650ns each) serializes
            # per-engine, and only sync/scalar/gpsimd can initiate DMAs —
            # fan the critical ones out across all three in need-order:
            # mproj wants m8+xb8[0], scores want k8 (quartered so scores
            # tile t only waits on its quarter via subtile deps).
            nc.scalar.dma_start(m8_s[:], m8d[:])
            nc.scalar.dma_start(xb8[:, :, 0:QBLK], xb8d[:, :, 0:QBLK])
            nc.scalar.dma_start(rq_s[:], rqd[:])
            KQ = LK // 4
            for q in range(2):
                nc.gpsimd.dma_start(k8[:, :, q * KQ:(q + 1) * KQ],
                                    k8d[:, :, q * KQ:(q + 1) * KQ])
            nc.sync.dma_start(k8[:, :, 2 * KQ:3 * KQ],
                              k8d[:, :, 2 * KQ:3 * KQ])
            nc.sync.dma_start(k8[:, :, 3 * KQ:4 * KQ],
                              k8d[:, :, 3 * KQ:4 * KQ])
            vc = (0, 7, 14, NKP)
            for h in range(3):
                nc.sync.dma_start(vab[:, vc[h]:vc[h + 1], :, :],
                                  vabd[:, vc[h]:vc[h + 1], :, :])
            for t in range(1, NQB):
                nc.sync.dma_start(xb8[:, :, t * QBLK:(t + 1) * QBLK],
                                  xb8d[:, :, t * QBLK:(t + 1) * QBLK])
            nc.sync.dma_start(n8_s[:], n8d[:])
            nc.vector.memset(zb[:], 0.0)
            masks.make_identity(nc, identb[:])

            def next_slot():
                return rgp.tile([128, 512], DT, tag="ring", name="ring")[:]

            def mproj(t, engines):
                # q8[t][:, co, :] = e4m3(0.25*(m8[:,:,co].T @ xb8_t) + 16r)
                for co in range(2):
                    ps = next_slot()
                    nc.tensor.matmul(
                        ps, m8_s[:, :, co, :],
                        xb8[:, :, t * QBLK:(t + 1) * QBLK],
                        perf_mode=DR, start=True, stop=True)
                    dst = q8_b[t][:, co, :]
                    if engines[co] == "A":
                        nc.scalar.activation(dst, ps, ActF.Identity,
                                             bias=rq_s[:, co:co + 1],
                                             scale=MPS)
                    else:
                        nc.vector.tensor_scalar(dst, ps, MPS,
                                                rq_s[:, co:co + 1],
                                                Alu.mult, Alu.add)

            def scores_single(qb, t, exb_list, eng):
                ps = next_slot()
                nc.tensor.matmul(
                    ps, k8[:, :, t * KT:(t + 1) * KT], q8_b[qb][:, :, :],
                    perf_mode=DR, start=True, stop=True)
                if t % 2 == 0:
                    exb_list[t // 2] = exp_pool.tile([128, 2, QBLK], I8,
                                                     tag="exb", name="exb")
                dst = exb_list[t // 2][:, t % 2, :]
                if eng == "A":
                    nc.scalar.activation(dst.bitcast(F8E5), ps, ActF.Exp,
                                         bias=zb[:], scale=1.0 / QSCALE)
                else:
                    nc.vector.tensor_scalar(dst, ps, A_SC, B_SC,
                                            Alu.mult, Alu.add)

            def av_mms(j, kps, exb_list, avst):
                if 0 in kps:
                    avst["av"] = avp.tile([128, 512], DT, tag="av",
                                          name="av")
                av = avst["av"]
                for kp in kps:
                    nc.tensor.matmul(
                        av[:, 0:VW],
                        exb_list[kp][:, :, j * 128:(j + 1) * 128]
                        .bitcast(F8E5),
                        vab[:, kp, :, :],
                        perf_mode=DR,
                        start=(kp == 0), stop=(kp == NKP - 1))

            # Epilogue stages are scheduled 2 steps apart so each stage's
            # cross-engine inputs are produced >1us earlier: neither PE nor
            # DVE blocks at its queue head on a fresh dependency.
            def epi_s0(j, qb, st):
                # Z is ~constant across q (rel std ~2.4%), so convert av
                # with a FIXED 2^-8 scale; the exact per-row 1/Z lands in
                # the final residual-add as a per-partition scalar.  This
                # keeps reciprocal off the PE transpose's critical path.
                rows = qb * QBLK + j * 128
                av = st["avst"]["av"]
                rec = elp.tile([128, 1], DT, tag="rec")
                nc.vector.reciprocal(rec[:], av[:, 256:257])
                avn = elp.tile([128, H], BF, tag="avn")
                nc.vector.tensor_scalar_mul(avn[:], av[:, 0:H], 1.0 / 256.0)
                res_t = elp.tile([128, H], F16, tag="res")
                nc.sync.dma_start(res_t[:], resd[rows:rows + 128, :])
                st["avn"], st["res_t"], st["rec"] = avn, res_t, rec

            def epi_s1(j, qb, st):
                avn = st["avn"]
                avnT = elp.tile([128, 2, 128], F8E4, tag="avnT")
                tps = next_slot()
                for c in range(2):
                    tp = tps[:, 64 * c:64 * (c + 1)].bitcast(BF)
                    nc.tensor.transpose(tp, avn[:, c * 128:(c + 1) * 128],
                                        identb[:])
                for c in range(2):
                    tp = tps[:, 64 * c:64 * (c + 1)].bitcast(BF)
                    nc.scalar.copy(avnT[:, c, :], tp)
                st["avnT"] = avnT

            def epi_s2(j, qb, st):
                rows = qb * QBLK + j * 128
                op = next_slot()[:, 0:H]
                nc.tensor.matmul(op, st["avnT"][:, :, :], n8_s[:, :, :],
                                 perf_mode=DR, start=True, stop=True)
                out_t = elp.tile([128, H], F16, tag="out")
                nc.vector.scalar_tensor_tensor(out_t[:], op, st["rec"][:],
                                               st["res_t"], Alu.mult, Alu.add)
                nc.sync.dma_start(out[rows:rows + 128, :], out_t[:])

            # ---- phase 1: M-proj + scores/exp for q-block 0 ----
            exb_all = [[None] * NKP for _ in range(NQB)]
            mproj(0, "AD")
            mproj(1, "DA")
            for t in range(NKT):
                scores_single(0, t, exb_all[0], PH1[t // 2])

            # ---- main schedule: flat global-step event list ----
            events = []  # (g, prio, fn)
            for qb in range(NQB):
                g0 = qb * NKP
                if qb + 1 < NQB:
                    for s in range(NKP):
                        # steps hosting an epi_s0 emit scores AFTER it, so
                        # avn lands ahead of those exps in the DVE queue
                        # and the PE transposes two steps later never wait
                        pr = 5 if s in (1, 6, 11, 16) else 0
                        events.append((g0 + s, pr,
                                       lambda s=s, qb=qb:
                                       [scores_single(qb + 1, 2 * s + h,
                                                      exb_all[qb + 1],
                                                      STD[s])
                                        for h in range(2)]))
                if qb + 2 < NQB:
                    events.append((g0 + 1, 1,
                                   lambda qb=qb: mproj(qb + 2, "AD")))
                for j in range(4):
                    avst = {}
                    for s in range(5):
                        events.append((g0 + 5 * j + 2 + s, 2,
                                       lambda j=j, s=s, qb=qb, avst=avst:
                                       av_mms(j, range(s * 4, s * 4 + 4),
                                              exb_all[qb], avst)))
                    st = {"avst": avst}
                    # last window: exp engines are idle, so the epilogue's
                    # cross-engine stages need no queue-hiding stagger
                    d1, d2 = (1, 2) if qb == NQB - 1 else (2, 4)
                    events.append((g0 + 5 * j + 6, 3,
                                   lambda j=j, qb=qb, st=st:
                                   epi_s0(j, qb, st)))
                    events.append((g0 + 5 * j + 6 + d1, 3,
                                   lambda j=j, qb=qb, st=st:
                                   epi_s1(j, qb, st)))
                    events.append((g0 + 5 * j + 6 + d2, 3,
                                   lambda j=j, qb=qb, st=st:
                                   epi_s2(j, qb, st)))
            events.sort(key=lambda e: (e[0], e[1]))
            for _, _, fn in events:
                fn()

    nc.compile()
    return nc


_nc_cache = None
last_results = None


def _get_nc():
    global _nc_cache
    if _nc_cache is None:
        _nc_cache = build()
    return _nc_cache


def _fmajor(w, np_dt):
    # [256, X] -> [128, 2, X] with row c*128+p -> [p, c]
    return np.ascontiguousarray(
        w.reshape(2, 128, -1).transpose(1, 0, 2).astype(np_dt))


def _vab_layout(xg):
    # [5120, 256] -> [128, NKP, 2, VW] e5m2: token t = (kp*2+c)*128+p,
    # cols 0:256 = xg row, col 256 = 1/16 (so Z accumulates as Z/16).
    v = np.zeros((NKP, 2, 128, VW), dtype=NP_E5)
    v[:, :, :, 0:F] = xg.reshape(NKP, 2, 128, F).astype(NP_E5)
    v[:, :, :, F] = np.float32(1.0 / 16.0)
    return np.ascontiguousarray(v.transpose(2, 0, 1, 3))


def kernel(**inputs):
    global last_results
    ab = np.ascontiguousarray(inputs["ab"], dtype=np.float32)
    ag = np.ascontiguousarray(inputs["ag"], dtype=np.float32)
    Wq = np.asarray(inputs["Wq"], dtype=np.float32)
    Wk = np.asarray(inputs["Wk"], dtype=np.float32)
    Wv = np.asarray(inputs["Wv"], dtype=np.float32)
    Wo = np.asarray(inputs["Wo"], dtype=np.float32)
    bq = np.asarray(inputs["bq"], dtype=np.float32)
    bv = np.asarray(inputs["bv"], dtype=np.float32)
    bo = np.asarray(inputs["bo"], dtype=np.float32)

    s = np.float32(1.0 / np.sqrt(np.float32(H)))
    M = s * (Wq @ Wk.T)                          # [256 f, 256 d]
    N = Wv @ Wo                                  # [256 d, 256 f]
    r = s * (bq @ Wk.T)                          # [256 d]; bk cancels
    cvec = bv @ Wo + bo                          # [256 f]

    m8 = np.ascontiguousarray(                   # [p, ci, co, m]
        (64.0 * M).reshape(2, 128, 2, 128).transpose(1, 0, 2, 3)
        .astype(NP_E4))
    n8 = _fmajor(16.0 * N, NP_E4)                # [p, c, f]
    rqd = np.ascontiguousarray((QSCALE * r).reshape(2, 128).T,
                               dtype=np.float32)

    in_maps = []
    for b in range(B):
        xb = ab[b].reshape(LQ, F)
        xg = ag[b].reshape(LK, F)
        in_maps.append({
            "xb8d": _fmajor(xb.T, NP_E4),
            "k8d": _fmajor(xg.T, NP_E4),
            "vabd": _vab_layout(xg),
            "resd": np.ascontiguousarray((xb + cvec[None, :])
                                         .astype(NP_F16)),
            "m8d": m8, "n8d": n8, "rqd": rqd,
        })

    nc = _get_nc()
    last_exc = None
    for _attempt in range(3):
        try:
            last_results = run_bass_kernel_spmd(
                nc, in_maps, core_ids=list(range(NCORES)))
            break
        except Exception as e:  # transient device flakes
            last_exc = e
    else:
        raise last_exc
    return np.stack([last_results.results[b]["out"].reshape(L, A, F)
                     for b in range(B)]).astype(np.float32)
